# revision 29
# baseline (speedup 1.0000x reference)
import os
import sys

import numpy as np

if "/opt/trn_rl_repo" not in sys.path:
    sys.path.insert(0, "/opt/trn_rl_repo")

N = 2048          # atoms
NCORES = 8
NL = N // NCORES  # 256 rows of plm per core
C = 16            # c_atom_pair
P = 128           # partitions
C_IN, C_ATOM = 390, 128

_CACHE = {}


def _build_program():
    import concourse.bacc as bacc
    import concourse.bass as bass
    import concourse.mybir as mybir
    import concourse.tile as tile

    dt = mybir.dt.float32
    Alu = mybir.AluOpType
    Act = mybir.ActivationFunctionType

    nc = bacc.Bacc("TRN2", target_bir_lowering=False, debug=False,
                   num_devices=NCORES)

    # ---- DRAM I/O (per-core shapes; values differ per core) ----
    posm_b = nc.dram_tensor("posm_b", [3, P, N], dt, kind="ExternalInput")
    uid_b = nc.dram_tensor("uid_b", [P, N], dt, kind="ExternalInput")
    pu = nc.dram_tensor("pu", [NL, 4], dt, kind="ExternalInput")
    depack = nc.dram_tensor("depack", [80, 4 * 64 * C], dt,
                            kind="ExternalInput")
    awt = nc.dram_tensor("awt", [8, 4 * 64 * C], dt, kind="ExternalInput")
    tail17 = nc.dram_tensor("tail17", [C + 1, NL], dt, kind="ExternalInput")
    eye = nc.dram_tensor("eye", [P, P], dt, kind="ExternalInput")
    fwpack = nc.dram_tensor("fwpack", [P, 4 * (NL + C_ATOM)], dt,
                            kind="ExternalInput")

    plm_o = nc.dram_tensor("plm_o", [NL, N, C], dt, kind="ExternalOutput")
    cl_o = nc.dram_tensor("cl_o", [NL, C_ATOM], dt, kind="ExternalOutput")

    def ap3(base, dims):
        # rebuild an AP with explicit free-dim [step,count] list
        return bass.AP(base.tensor, base.offset, [list(base.ap[0])] + dims)

    with tile.TileContext(nc) as tc:
        with (
            tc.tile_pool(name="const", bufs=1) as cp,
            tc.tile_pool(name="lbp", bufs=2) as lp,
            tc.tile_pool(name="plane", bufs=1) as pp,
            tc.tile_pool(name="outp", bufs=3) as op_,
            tc.tile_pool(name="psum", bufs=2, space="PSUM") as ps,
        ):
            # ---- load constants ----
            t_posm = []
            for k in range(3):
                t = cp.tile([P, N], dt, tag=f"t_posm{k}")
                nc.sync.dma_start(t[:], posm_b[k, :, :])
                t_posm.append(t)
            t_uidb = cp.tile([P, N], dt, tag="t_uidb")
            nc.sync.dma_start(t_uidb[:], uid_b[:])
            t_eye = cp.tile([P, P], dt, tag="t_eye")
            nc.sync.dma_start(t_eye[:], eye[:])
            # two ping-pong rhs tiles: rows 0:64 = kron(I64,W_inv),
            # rows 64:80 = tiled eye16, row 80 = per-block aw row (per-iter DMA)
            t_rhs = []
            for r in range(2):
                t = cp.tile([81, 4 * 64 * C], dt, tag=f"t_rhs{r}")
                nc.sync.dma_start(t[0:80, :], depack[:])
                t_rhs.append(t)
            t_aw = cp.tile([8, 4 * 64 * C], dt, tag="t_aw")
            nc.sync.dma_start(t_aw[:], awt[:])
            t_fw = cp.tile([P, 4 * (NL + C_ATOM)], dt, tag="t_fw")
            nc.sync.dma_start(t_fw[:], fwpack[:])
            S = NL + C_ATOM

            # ---- cl = feats @ W_feats (tiny) ----
            for ab in range(NL // P):
                ps_cl = ps.tile([P, C_ATOM], dt, tag="pmain")
                for k in range(4):
                    nc.tensor.matmul(
                        ps_cl[:],
                        t_fw[:, k * S + ab * P:k * S + (ab + 1) * P],
                        t_fw[:, k * S + NL:k * S + NL + C_ATOM],
                        start=(k == 0),
                        stop=(k == 3),
                    )
                sb_cl = op_.tile([P, C_ATOM], dt, tag="sb_cl")
                nc.scalar.copy(sb_cl[:], ps_cl[:])
                nc.sync.dma_start(cl_o[ab * P:(ab + 1) * P, :], sb_cl[:])

            # ---- plm ----
            for lb in range(NL // P):
                # per-partition scalars for this l-block (pos x/y/z, uid)
                t_pu = lp.tile([P, 4], dt, tag="t_pu")
                nc.sync.dma_start(t_pu[:], pu[lb * P:(lb + 1) * P, :])

                # [l, m] planes: d2 -> d -> inv_d ; u ; w
                t_d = pp.tile([P, N], dt, tag="t_d")
                t_s = pp.tile([P, N], dt, tag="t_s")
                t_acc = pp.tile([P, N], dt, tag="t_acc")
                for k in range(3):
                    nc.vector.tensor_scalar_sub(
                        t_d[:], t_posm[k][:], t_pu[:, k:k + 1]
                    )
                    if k == 0:
                        nc.gpsimd.tensor_mul(t_acc[:], t_d[:], t_d[:])
                    else:
                        nc.gpsimd.tensor_mul(t_s[:], t_d[:], t_d[:])
                        nc.gpsimd.tensor_add(t_acc[:], t_acc[:], t_s[:])
                # d = exp(0.5*ln(d2)) ; inv_d = exp(-ln(1+d))  (diagonal: exact 1.0)
                nc.scalar.activation(t_s[:], t_acc[:], Act.Ln)
                nc.scalar.activation(t_acc[:], t_s[:], Act.Exp, scale=0.5)
                nc.scalar.activation(t_s[:], t_acc[:], Act.Ln, bias=1.0)
                nc.scalar.activation(t_acc[:], t_s[:], Act.Exp, scale=-1.0)

                t_u = lp.tile([P, N], dt, tag="t_u")
                nc.vector.tensor_scalar(
                    out=t_u[:], in0=t_uidb[:], scalar1=t_pu[:, 3:4],
                    scalar2=None, op0=Alu.is_equal,
                )
                t_w = lp.tile([P, N], dt, tag="t_w")
                nc.gpsimd.tensor_mul(t_w[:], t_u[:], t_acc[:])

                # fused single-matmul blocks of 64 m:
                # lhsT[0:64]  = w[l, m64].T   (PE transpose)
                # lhsT[64:80] = -a[l,:].T ; lhsT[80] = ones
                # rhs = [kron(I64,W_inv); tiled eye16; aw row]  K=81
                t_lhs = []
                for h in range(2):
                    t = lp.tile([81, P], dt, tag=f"t_lhs{h}")
                    nc.sync.dma_start(
                        t[64:81, :], tail17[:, lb * P:(lb + 1) * P])
                    t_lhs.append(t)
                for mb2 in range(N // 256):        # 4 blocks of 64 m each
                    t_out = op_.tile([P, 256 * C], dt, tag="t_out")
                    grp = mb2
                    rhs = t_rhs[grp % 2]
                    nc.sync.dma_start(rhs[80:81, :], t_aw[grp:grp + 1, :])
                    for h in range(4):
                        blk = mb2 * 4 + h
                        m0 = blk * 64
                        lhs = t_lhs[h % 2]
                        ps_t = ps.tile([64, P], dt, tag="ptr")
                        nc.tensor.transpose(
                            ps_t[:], t_w[:, m0:m0 + 64], t_eye[:]
                        )
                        nc.vector.tensor_copy(lhs[0:64, :], ps_t[:])
                        ps_m = ps.tile([P, 64 * C], dt, tag="pmain")
                        for j in range(2):
                            sl = slice(h * 64 * C + j * 512,
                                       h * 64 * C + (j + 1) * 512)
                            nc.tensor.matmul(
                                ps_m[:, j * 512:(j + 1) * 512],
                                lhs[:], rhs[:, sl],
                                start=True, stop=True,
                            )
                        ub = t_u[:, m0:m0 + 64]
                        u3 = ap3(ub, [list(ub.ap[1]), [0, C]])
                        p3 = ap3(ps_m[:, :], [[C, 64], [1, C]])
                        ob = t_out[:, h * 64 * C:(h + 1) * 64 * C]
                        o3 = ap3(ob, [[C, 64], [1, C]])
                        nc.vector.tensor_mul(o3, p3, u3)
                    eng = nc.sync if mb2 % 2 == 0 else nc.scalar
                    eng.dma_start(
                        plm_o[lb * P:(lb + 1) * P, mb2 * 256:(mb2 + 1) * 256, :],
                        t_out[:],
                    )
    nc.compile()
    return nc


def _prep_inputs(ref_pos, ref_mask, ref_element, ref_charge, ref_atom_name_chars,
                 ref_space_uid, W_feats, W_off, W_inv, W_mask):
    f4 = np.float32
    pos = np.asarray(ref_pos, f4)[0]                      # [N,3]
    uidf = np.asarray(ref_space_uid)[0].astype(f4)        # [N]
    a = pos @ np.asarray(W_off, f4)                       # [N,C]
    aw = a + np.asarray(W_mask, f4)[0]                    # [N,C]
    awt = np.ascontiguousarray(aw.reshape(-1).reshape(N // 256, 256 * C))
    D = np.kron(np.eye(64, dtype=f4), np.asarray(W_inv, f4))  # [64, 64*C]
    E = np.tile(np.eye(C, dtype=f4), (1, 64))                 # [C, 64*C]
    depack = np.ascontiguousarray(
        np.tile(np.concatenate([D, E], axis=0), (1, 4)))      # [80, 4*64*C]
    feats = np.concatenate([
        pos,
        np.asarray(ref_mask, f4)[0][:, None],
        np.asarray(ref_element, f4)[0],
        np.asarray(ref_charge, f4)[0][:, None],
        np.asarray(ref_atom_name_chars, f4)[0].reshape(N, 256),
        uidf[:, None],
    ], axis=1)                                            # [N,390]
    fTp = np.zeros((512, N), f4)
    fTp[:C_IN] = feats.T
    wfp = np.zeros((512, C_ATOM), f4)
    wfp[:C_IN] = np.asarray(W_feats, f4)
    posm_b = np.ascontiguousarray(
        np.broadcast_to(pos.T[:, None, :], (3, P, N)), f4)
    uid_b = np.ascontiguousarray(np.broadcast_to(uidf[None, :], (P, N)), f4)
    eye = np.eye(P, dtype=f4)

    in_maps = []
    for i in range(NCORES):
        l0 = i * NL
        fT4 = fTp[:, l0:l0 + NL].reshape(4, P, NL)        # [4,128,256]
        wf4 = wfp.reshape(4, P, C_ATOM)                   # [4,128,128]
        fwpack = np.ascontiguousarray(np.concatenate(
            [np.concatenate([fT4[k], wf4[k]], axis=1) for k in range(4)],
            axis=1))                                      # [128, 4*384]
        in_maps.append({
            "posm_b": posm_b,
            "uid_b": uid_b,
            "pu": np.ascontiguousarray(np.concatenate(
                [pos[l0:l0 + NL], uidf[l0:l0 + NL, None]], axis=1)),
            "awt": awt,
            "tail17": np.ascontiguousarray(np.concatenate(
                [(-a[l0:l0 + NL]).T, np.ones((1, NL), f4)], axis=0)),
            "depack": depack,
            "eye": eye,
            "fwpack": fwpack,
        })
    return in_maps


def kernel(**inputs):
    from concourse.bass_utils import run_bass_kernel_spmd

    if "nc" not in _CACHE:
        _CACHE["nc"] = _build_program()
    nc = _CACHE["nc"]
    in_maps = _prep_inputs(**inputs)
    res = run_bass_kernel_spmd(nc, in_maps, list(range(NCORES))).results
    cl = np.concatenate([r["cl_o"] for r in res], axis=0)[None]          # [1,N,128]
    plm = np.concatenate([r["plm_o"] for r in res], axis=0)[None]        # [1,N,N,C]
    return cl.astype(np.float32), plm.astype(np.float32)


# revision 32
# speedup vs baseline: 64720.4163x; 64720.4163x over previous
import os
import sys

import numpy as np

if "/opt/trn_rl_repo" not in sys.path:
    sys.path.insert(0, "/opt/trn_rl_repo")

N = 2048          # atoms
NCORES = 8
NL = N // NCORES  # 256 rows of plm per core
C = 16            # c_atom_pair
P = 128           # partitions
C_IN, C_ATOM = 390, 128

_CACHE = {}


def _build_program():
    import concourse.bacc as bacc
    import concourse.bass as bass
    import concourse.mybir as mybir
    import concourse.tile as tile

    dt = mybir.dt.float32
    Alu = mybir.AluOpType
    Act = mybir.ActivationFunctionType

    nc = bacc.Bacc("TRN2", target_bir_lowering=False, debug=False,
                   num_devices=NCORES)

    # ---- DRAM I/O (per-core shapes; values differ per core) ----
    posm_b = nc.dram_tensor("posm_b", [3, P, N], dt, kind="ExternalInput")
    uid_b = nc.dram_tensor("uid_b", [P, N], dt, kind="ExternalInput")
    pu = nc.dram_tensor("pu", [NL, 4], dt, kind="ExternalInput")
    depack = nc.dram_tensor("depack", [80, 4 * 64 * C], dt,
                            kind="ExternalInput")
    awt = nc.dram_tensor("awt", [8, 4 * 64 * C], dt, kind="ExternalInput")
    tail17 = nc.dram_tensor("tail17", [C + 1, NL], dt, kind="ExternalInput")
    eye = nc.dram_tensor("eye", [P, P], dt, kind="ExternalInput")
    fwpack = nc.dram_tensor("fwpack", [P, 4 * (NL + C_ATOM)], dt,
                            kind="ExternalInput")

    plm_o = nc.dram_tensor("plm_o", [NL, N, C], dt, kind="ExternalOutput")
    cl_o = nc.dram_tensor("cl_o", [NL, C_ATOM], dt, kind="ExternalOutput")

    def ap3(base, dims):
        # rebuild an AP with explicit free-dim [step,count] list
        return bass.AP(base.tensor, base.offset, [list(base.ap[0])] + dims)

    with tile.TileContext(nc) as tc:
        with (
            tc.tile_pool(name="const", bufs=1) as cp,
            tc.tile_pool(name="lbp", bufs=2) as lp,
            tc.tile_pool(name="plane", bufs=1) as pp,
            tc.tile_pool(name="outp", bufs=3) as op_,
            tc.tile_pool(name="psum", bufs=2, space="PSUM") as ps,
        ):
            # ---- load constants ----
            t_posm = []
            for k in range(3):
                t = cp.tile([P, N], dt, tag=f"t_posm{k}")
                nc.sync.dma_start(t[:], posm_b[k, :, :])
                t_posm.append(t)
            t_uidb = cp.tile([P, N], dt, tag="t_uidb")
            nc.sync.dma_start(t_uidb[:], uid_b[:])
            t_eye = cp.tile([P, P], dt, tag="t_eye")
            nc.sync.dma_start(t_eye[:], eye[:])
            # two ping-pong rhs tiles: rows 0:64 = kron(I64,W_inv),
            # rows 64:80 = tiled eye16, row 80 = per-block aw row (per-iter DMA)
            t_rhs = []
            for r in range(2):
                t = cp.tile([81, 4 * 64 * C], dt, tag=f"t_rhs{r}")
                nc.sync.dma_start(t[0:80, :], depack[:])
                t_rhs.append(t)
            t_aw = cp.tile([8, 4 * 64 * C], dt, tag="t_aw")
            nc.sync.dma_start(t_aw[:], awt[:])
            t_fw = cp.tile([P, 4 * (NL + C_ATOM)], dt, tag="t_fw")
            nc.sync.dma_start(t_fw[:], fwpack[:])
            S = NL + C_ATOM

            # ---- cl = feats @ W_feats (tiny) ----
            for ab in range(NL // P):
                ps_cl = ps.tile([P, C_ATOM], dt, tag="pmain", bufs=3)
                for k in range(4):
                    nc.tensor.matmul(
                        ps_cl[:],
                        t_fw[:, k * S + ab * P:k * S + (ab + 1) * P],
                        t_fw[:, k * S + NL:k * S + NL + C_ATOM],
                        start=(k == 0),
                        stop=(k == 3),
                    )
                sb_cl = op_.tile([P, C_ATOM], dt, tag="sb_cl")
                nc.scalar.copy(sb_cl[:], ps_cl[:])
                nc.sync.dma_start(cl_o[ab * P:(ab + 1) * P, :], sb_cl[:])

            # ---- plm ----
            for lb in range(NL // P):
                # per-partition scalars for this l-block (pos x/y/z, uid)
                t_pu = lp.tile([P, 4], dt, tag="t_pu")
                nc.sync.dma_start(t_pu[:], pu[lb * P:(lb + 1) * P, :])

                # [l, m] planes: d2 -> d -> inv_d ; u ; w
                t_d = pp.tile([P, N], dt, tag="t_d")
                t_s = pp.tile([P, N], dt, tag="t_s")
                t_acc = pp.tile([P, N], dt, tag="t_acc")
                for k in range(3):
                    nc.gpsimd.tensor_scalar_sub(
                        t_d[:], t_posm[k][:], t_pu[:, k:k + 1]
                    )
                    if k == 0:
                        nc.gpsimd.tensor_mul(t_acc[:], t_d[:], t_d[:])
                    else:
                        nc.gpsimd.tensor_mul(t_s[:], t_d[:], t_d[:])
                        nc.gpsimd.tensor_add(t_acc[:], t_acc[:], t_s[:])
                # d = exp(0.5*ln(d2)) ; inv_d = exp(-ln(1+d))  (diagonal: exact 1.0)
                nc.scalar.activation(t_s[:], t_acc[:], Act.Ln)
                nc.scalar.activation(t_acc[:], t_s[:], Act.Exp, scale=0.5)
                nc.scalar.activation(t_s[:], t_acc[:], Act.Ln, bias=1.0)
                nc.scalar.activation(t_acc[:], t_s[:], Act.Exp, scale=-1.0)

                t_u = lp.tile([P, N], dt, tag="t_u")
                nc.gpsimd.tensor_scalar(
                    out=t_u[:], in0=t_uidb[:], scalar1=t_pu[:, 3:4],
                    scalar2=None, op0=Alu.is_equal,
                )
                t_w = lp.tile([P, N], dt, tag="t_w")
                nc.gpsimd.tensor_mul(t_w[:], t_u[:], t_acc[:])

                # fused single-matmul blocks of 64 m:
                # lhsT[0:64]  = w[l, m64].T   (PE transpose)
                # lhsT[64:80] = -a[l,:].T ; lhsT[80] = ones
                # rhs = [kron(I64,W_inv); tiled eye16; aw row]  K=81
                t_lhs = []
                for h in range(2):
                    t = lp.tile([81, P], dt, tag=f"t_lhs{h}")
                    nc.sync.dma_start(
                        t[64:81, :], tail17[:, lb * P:(lb + 1) * P])
                    t_lhs.append(t)
                for mb2 in range(N // 256):        # 4 blocks of 64 m each
                    t_out = op_.tile([P, 256 * C], dt, tag="t_out")
                    grp = mb2
                    rhs = t_rhs[grp % 2]
                    nc.sync.dma_start(rhs[80:81, :], t_aw[grp:grp + 1, :])
                    for h in range(4):
                        blk = mb2 * 4 + h
                        m0 = blk * 64
                        lhs = t_lhs[h % 2]
                        ps_t = ps.tile([64, P], dt, tag="ptr", bufs=2)
                        nc.tensor.transpose(
                            ps_t[:], t_w[:, m0:m0 + 64], t_eye[:]
                        )
                        nc.vector.tensor_copy(lhs[0:64, :], ps_t[:])
                        ps_m = ps.tile([P, 64 * C], dt, tag="pmain", bufs=3)
                        for j in range(2):
                            sl = slice(h * 64 * C + j * 512,
                                       h * 64 * C + (j + 1) * 512)
                            nc.tensor.matmul(
                                ps_m[:, j * 512:(j + 1) * 512],
                                lhs[:], rhs[:, sl],
                                start=True, stop=True,
                            )
                        ub = t_u[:, m0:m0 + 64]
                        u3 = ap3(ub, [list(ub.ap[1]), [0, C]])
                        p3 = ap3(ps_m[:, :], [[C, 64], [1, C]])
                        ob = t_out[:, h * 64 * C:(h + 1) * 64 * C]
                        o3 = ap3(ob, [[C, 64], [1, C]])
                        nc.vector.tensor_mul(o3, p3, u3)
                    eng = nc.sync if mb2 % 2 == 0 else nc.scalar
                    eng.dma_start(
                        plm_o[lb * P:(lb + 1) * P, mb2 * 256:(mb2 + 1) * 256, :],
                        t_out[:],
                    )
    nc.compile()
    return nc


def _prep_inputs(ref_pos, ref_mask, ref_element, ref_charge, ref_atom_name_chars,
                 ref_space_uid, W_feats, W_off, W_inv, W_mask):
    f4 = np.float32
    pos = np.asarray(ref_pos, f4)[0]                      # [N,3]
    uidf = np.asarray(ref_space_uid)[0].astype(f4)        # [N]
    a = pos @ np.asarray(W_off, f4)                       # [N,C]
    aw = a + np.asarray(W_mask, f4)[0]                    # [N,C]
    awt = np.ascontiguousarray(aw.reshape(-1).reshape(N // 256, 256 * C))
    D = np.kron(np.eye(64, dtype=f4), np.asarray(W_inv, f4))  # [64, 64*C]
    E = np.tile(np.eye(C, dtype=f4), (1, 64))                 # [C, 64*C]
    depack = np.ascontiguousarray(
        np.tile(np.concatenate([D, E], axis=0), (1, 4)))      # [80, 4*64*C]
    feats = np.concatenate([
        pos,
        np.asarray(ref_mask, f4)[0][:, None],
        np.asarray(ref_element, f4)[0],
        np.asarray(ref_charge, f4)[0][:, None],
        np.asarray(ref_atom_name_chars, f4)[0].reshape(N, 256),
        uidf[:, None],
    ], axis=1)                                            # [N,390]
    fTp = np.zeros((512, N), f4)
    fTp[:C_IN] = feats.T
    wfp = np.zeros((512, C_ATOM), f4)
    wfp[:C_IN] = np.asarray(W_feats, f4)
    posm_b = np.ascontiguousarray(
        np.broadcast_to(pos.T[:, None, :], (3, P, N)), f4)
    uid_b = np.ascontiguousarray(np.broadcast_to(uidf[None, :], (P, N)), f4)
    eye = np.eye(P, dtype=f4)

    in_maps = []
    for i in range(NCORES):
        l0 = i * NL
        fT4 = fTp[:, l0:l0 + NL].reshape(4, P, NL)        # [4,128,256]
        wf4 = wfp.reshape(4, P, C_ATOM)                   # [4,128,128]
        fwpack = np.ascontiguousarray(np.concatenate(
            [np.concatenate([fT4[k], wf4[k]], axis=1) for k in range(4)],
            axis=1))                                      # [128, 4*384]
        in_maps.append({
            "posm_b": posm_b,
            "uid_b": uid_b,
            "pu": np.ascontiguousarray(np.concatenate(
                [pos[l0:l0 + NL], uidf[l0:l0 + NL, None]], axis=1)),
            "awt": awt,
            "tail17": np.ascontiguousarray(np.concatenate(
                [(-a[l0:l0 + NL]).T, np.ones((1, NL), f4)], axis=0)),
            "depack": depack,
            "eye": eye,
            "fwpack": fwpack,
        })
    return in_maps


def kernel(**inputs):
    from concourse.bass_utils import run_bass_kernel_spmd

    if "nc" not in _CACHE:
        _CACHE["nc"] = _build_program()
    nc = _CACHE["nc"]
    in_maps = _prep_inputs(**inputs)
    res = run_bass_kernel_spmd(nc, in_maps, list(range(NCORES))).results
    cl = np.concatenate([r["cl_o"] for r in res], axis=0)[None]          # [1,N,128]
    plm = np.concatenate([r["plm_o"] for r in res], axis=0)[None]        # [1,N,N,C]
    return cl.astype(np.float32), plm.astype(np.float32)
